# revision 40
# baseline (speedup 1.0000x reference)
"""Distributed Bass kernel for nn_AttentionLayer (B=2, S=2048, D=1024, H=16).

Sharding: tensor-parallel over heads. Core c owns heads {2c, 2c+1} (128 of the
1024 hidden dims). v3: fully software-pipelined emission so every engine's
queue streams without phase barriers:

  - K/Q projections tc-outer/dc-inner (2 PSUM banks), V projected directly in
    natural [token, dim] layout (stationary = X^T chunk, moving = Wv) with the
    bias applied as a rank-1 ones-row matmul.
  - attention starts as soon as the first 512-token chunk of K/V/Q for batch 0
    is projected; the remaining projections (incl. all of batch 1) are fed
    into the attention emission stream by a cost-budgeted thunk feeder with
    named milestones that guarantee data is emitted before it is consumed.
  - attention per 512-token block: transposed scores (two heads row-packed
    into one [128,1024] PSUM tile), one exp per st on ACT, exp(bias) multiply
    on DVE, ones-augmented PV accumulation; scores emitted one st ahead.
  - per-batch chunked AllToAll (bf16 + bitcast-packed f32 denominators);
    batch-0's output projection runs under batch-1 attention. Host picks
    out_t0 for cores 0-3, out_t1 for cores 4-7.
  - bias/x DMAs ride the sync queue so the collectives (gpsimd queue) never
    block them; PE warmup stream at t=0 lifts the HAM clock gate early.
"""

import sys
from collections import deque

import numpy as np

for _p in ("/opt/trn_rl_repo",):
    if _p not in sys.path:
        sys.path.insert(0, _p)

import ml_dtypes

BF = ml_dtypes.bfloat16

B, S, D, H = 2, 2048, 1024, 16
DK = D // H            # 64
NCORES = 8
T = B * S              # 4096
OC = D // NCORES       # 128 hidden dims per core (2 heads)
TSL = T // NCORES      # 512 token slice per core after AllToAll
NST = S // 128         # 16 kv chunks per batch

_CACHE = {}


def _build_nc():
    import concourse.bass as bass
    import concourse.mybir as mybir
    import concourse.tile as tile
    from concourse import bacc

    f32 = mybir.dt.float32
    bf16 = mybir.dt.bfloat16
    AF = mybir.ActivationFunctionType
    MULT = mybir.AluOpType.mult

    nc = bacc.Bacc(
        "TRN2",
        target_bir_lowering=False,
        debug=False,
        num_devices=NCORES,
    )

    # ---- kernel I/O ----
    # host pre-arranges every tensor so each DMA is contiguous per
    # partition (descriptor-issue time on the engine queues dominates
    # scattered transfers)
    xq = nc.dram_tensor("xq_t", [8, 128, 8, 512], bf16, kind="ExternalInput")
    xk = nc.dram_tensor("xk_t", [8, 128, 8, 512], bf16, kind="ExternalInput")
    xv = nc.dram_tensor("xv_t", [8, 128, 8, 512], bf16, kind="ExternalInput")
    ebias_t = nc.dram_tensor("ebias_t", [B, 4, 128, NST, 512], bf16,
                             kind="ExternalInput")
    wq_t = nc.dram_tensor("wq_t", [128, 8, OC], bf16, kind="ExternalInput")
    wk_t = nc.dram_tensor("wk_t", [128, 8, OC], bf16, kind="ExternalInput")
    wv_t = nc.dram_tensor("wv_t", [128, 8, OC], bf16, kind="ExternalInput")
    wo_t = nc.dram_tensor("wo_t", [128, 8, D], bf16, kind="ExternalInput")
    bq_d = nc.dram_tensor("bq_r", [1, OC], bf16, kind="ExternalInput")
    bk_d = nc.dram_tensor("bk_r", [1, OC], bf16, kind="ExternalInput")
    bv_d = nc.dram_tensor("bv_r", [1, OC], bf16, kind="ExternalInput")
    bo_d = nc.dram_tensor("bo_f", [128, 8], f32, kind="ExternalInput")
    sel_d = nc.dram_tensor("sel", [H, D], bf16, kind="ExternalInput")

    kt_out = nc.dram_tensor("kt_out", [OC, T], bf16, kind="ExternalOutput")
    v_out = nc.dram_tensor("v_out", [128, 32, 130], bf16, kind="ExternalOutput")
    out_t0 = nc.dram_tensor("out_t0", [D, TSL], bf16, kind="ExternalOutput")
    out_t1a = nc.dram_tensor("out_t1a", [D, TSL], bf16, kind="ExternalOutput")
    out_t1b = nc.dram_tensor("out_t1b", [D, TSL], bf16, kind="ExternalOutput")

    rg = [list(range(NCORES))]

    class Feeder:
        """Cost-budgeted emission queue with named milestones.

        Items: (cost_ns, name|None, fn) or ("expand", fn_returning_items).
        """

        def __init__(self):
            self.q = deque()
            self.done = set()

        def add(self, *items):
            self.q.extend(items)

        def _pop_one(self):
            item = self.q.popleft()
            if item[0] == "expand":
                for sub in reversed(item[1]()):
                    self.q.appendleft(sub)
                return 0
            cost, name, fn = item
            fn()
            if name:
                self.done.add(name)
            return cost

        def feed(self, budget_ns):
            spent = 0
            while self.q and spent < budget_ns:
                spent += self._pop_one()

        def require(self, *names):
            for name in names:
                while name not in self.done:
                    if not self.q:
                        raise RuntimeError(f"milestone {name} unreachable")
                    self._pop_one()

        def drain(self):
            while self.q:
                self._pop_one()

    with tile.TileContext(nc) as tc:
        with tc.tile_pool(name="persist", bufs=1) as pp, \
             tc.tile_pool(name="dramp", bufs=1, space="DRAM") as dramp, \
             tc.tile_pool(name="xin", bufs=5) as xinp, \
             tc.tile_pool(name="biasp", bufs=2) as biasp, \
             tc.tile_pool(name="prsb", bufs=5) as prsb, \
             tc.tile_pool(name="evsb", bufs=2) as evsb, \
             tc.tile_pool(name="pr_ps", bufs=2, space="PSUM") as prps, \
             tc.tile_pool(name="sc_ps", bufs=2, space="PSUM") as scps, \
             tc.tile_pool(name="pv_ps", bufs=1, space="PSUM") as pvps:

            # ---------------- persistent tiles ----------------
            # warmup stationary: memset (no DMA dependency -> PE busy at t~1us)
            warm_sb = pp.tile([128, 128], bf16)
            nc.vector.memset(warm_sb[:], 0.25)
            wk_sb = pp.tile([128, 8, OC], bf16)
            nc.sync.dma_start(wk_sb[:], wk_t.ap())
            bk_sb = pp.tile([1, OC], bf16)
            nc.sync.dma_start(bk_sb[:], bk_d.ap())
            wv_sb = pp.tile([128, 8, OC], bf16)
            nc.scalar.dma_start(wv_sb[:], wv_t.ap())
            bv_sb = pp.tile([1, OC], bf16)
            nc.scalar.dma_start(bv_sb[:], bv_d.ap())
            ones_sb = pp.tile([1, 512], bf16)
            nc.vector.memset(ones_sb[:], 1.0)

            # wq/bq DMAs issued in the emission head (after xk00) so the
            # first K-projection chunk lands as early as possible
            wq_sb = pp.tile([128, 8, OC], bf16)
            bq_sb = pp.tile([1, OC], bf16)

            # loaded lazily (during attention b0) via feeder
            wo_sb = pp.tile([128, 8, D], bf16)
            bo_sb = pp.tile([128, 8], f32)
            sel_sb = pp.tile([H, D], bf16)

            # persistent activations
            qt_sb = pp.tile([OC, T], bf16)
            kt_sb = pp.tile([OC, T], bf16)
            # v natural layout: [vA(64) | ones | vB(64) | ones] per 128-tok chunk
            v_sb = pp.tile([128, T // 128, 130], bf16)
            nc.vector.memset(v_sb[:, :, 64:65], 1.0)
            nc.vector.memset(v_sb[:, :, 129:130], 1.0)

            # outproj gather buffers (one out_acc per outproj pass so the
            # passes never serialize on WAR over the staging tile)
            attn_rv = pp.tile([128, 8, TSL], bf16)
            attn_n = pp.tile([128, 8, TSL], bf16)
            den_rv = pp.tile([H, TSL], f32)
            rcp_sb = pp.tile([H, TSL], f32)
            rcp_bf = pp.tile([H, TSL], bf16)
            out_accs = [pp.tile([128, 8, TSL], bf16, name=f"oacc{i}")
                        for i in range(3)]

            # collective bounce: rows [atA 0:64 | atB 64:128 | dens 128:132]
            # three collectives: b0 blocks 0-3 (slots 0-3), b1 blocks 0-1
            # (slots 4-5), b1 blocks 2-3 (slots 6-7); host picks out_t0 on
            # cores 0-3, out_t1a on 4-5, out_t1b on 6-7
            a2a_in = [dramp.tile([NCORES, 132, TSL], bf16, name=f"a2ai{b}")
                      for b in range(3)]
            a2a_out = [dramp.tile([NCORES, 132, TSL], bf16, name=f"a2ao{b}")
                       for b in range(3)]

            # ---------------- PE warmup (lift HAM clock gate) ----------------
            wm = scps.tile([128, 1024], f32, tag="ps_sc", name="warm")
            for i in range(24):
                nc.tensor.matmul(wm[:, 0:128], warm_sb[:], warm_sb[:],
                                 start=True, stop=True)
            wm_rd = evsb.tile([1, 8], f32, tag="wmrd")
            nc.vector.tensor_copy(wm_rd[:], wm[0:1, 0:8])
            # ACT exp table warm-up
            scr = evsb.tile([128, 16], bf16, tag="scr")
            nc.vector.memset(scr[:], 0.0)
            scr2 = evsb.tile([128, 16], bf16, tag="scr2")
            nc.scalar.activation(scr2[:], scr[:], AF.Exp)

            # ---------------- projection emitters ----------------
            def x_dma(xd, b, tcg, eng=None):
                xt = xinp.tile([128, 8, 512], bf16, tag="xch")
                (eng or nc.sync).dma_start(xt[:], xd.ap()[b * 4 + tcg])
                return xt

            def kq_proj_tc(xt, w_sb, b_sb, dst_sb, b, tcg, name):
                ops = []
                ps = prps.tile([128, 512], f32, tag="ps_pr")

                def mm(dc):
                    def f():
                        nc.tensor.matmul(ps[:], w_sb[:, dc, :], xt[:, dc, :],
                                         start=(dc == 0), stop=False)
                        if dc == 7:
                            # bias as a rank-1 matmul: b_row^T (x) ones
                            nc.tensor.matmul(ps[:], b_sb[:], ones_sb[:],
                                             start=False, stop=True)
                    return f
                ops.extend((220, None, mm(dc)) for dc in range(8))

                def evac():
                    sl = slice(b * S + tcg * 512, b * S + (tcg + 1) * 512)
                    nc.vector.tensor_copy(dst_sb[:, sl], ps[:])
                ops.append((0, name, evac))
                return ops

            def v_proj_tc(xt, b, tcg, name):
                ops = []
                ps = prps.tile([128, 512], f32, tag="ps_pr")
                for sub in range(4):
                    dst = ps[:, sub * 128:(sub + 1) * 128]

                    def mm(dc, dst=dst, sub=sub):
                        def f():
                            nc.tensor.matmul(
                                dst, xt[:, dc, sub * 128:(sub + 1) * 128],
                                wv_sb[:, dc, :], start=(dc == 0), stop=False)
                            if dc == 7:
                                nc.tensor.matmul(dst, ones_sb[:, 0:128],
                                                 bv_sb[:],
                                                 start=False, stop=True)
                        return f
                    ops.extend((180, None, mm(dc)) for dc in range(8))

                def evac():
                    for sub in range(4):
                        stg = b * NST + tcg * 4 + sub
                        src = ps[:, sub * 128:(sub + 1) * 128]
                        nc.vector.tensor_copy(v_sb[:, stg, 0:64], src[:, 0:64])
                        nc.vector.tensor_copy(v_sb[:, stg, 65:129], src[:, 64:128])
                ops.append((0, name, evac))
                return ops

            def tc_thunks(kind, b, tcg):
                """DMA thunk + expand-group for one 512-token projection."""
                box = {}
                xd, emit = {
                    "K": (xk, lambda: kq_proj_tc(box["t"], wk_sb, bk_sb, kt_sb,
                                                 b, tcg, f"K{b}{tcg}")),
                    "Q": (xq, lambda: kq_proj_tc(box["t"], wq_sb, bq_sb, qt_sb,
                                                 b, tcg, f"Q{b}{tcg}")),
                    "V": (xv, lambda: v_proj_tc(box["t"], b, tcg, f"V{b}{tcg}")),
                }[kind]
                # the first post-head batch-0 chunks ride the scalar queue
                # (idle until the exp stream begins); everything else stays
                # on sync — descriptor issue is cheap with the contiguous
                # host layouts, and gpsimd would block behind the collective
                if tcg == 1 and kind in ("K", "V") and b == 0:
                    eng = nc.scalar
                else:
                    eng = None
                dma = (0, None,
                       lambda: box.__setitem__("t", x_dma(xd, b, tcg, eng)))
                return [dma, ("expand", emit)]

            feeder = Feeder()

            # ---------------- attention block ----------------
            bias_q = {}
            at_q = {}

            def prefetch_bias(b, tcc, eng=None, pieces=1):
                bias_t = biasp.tile([128, NST, 512], bf16, tag="bias")
                np_ = NST // pieces
                for h in range(pieces):
                    (eng or nc.sync).dma_start(
                        bias_t[:, h * np_:(h + 1) * np_, :],
                        ebias_t.ap()[b, tcc, :, h * np_:(h + 1) * np_, :])
                bias_q[(b, tcc)] = bias_t

            def attn_block(b, tcc, budget, reqs=None, reqs_pv=None,
                           prefetch=None):
                j = b * 4 + tcc
                tg = b * S + tcc * 512
                bias_t = bias_q.pop((b, tcc))
                if prefetch is not None:
                    prefetch_bias(*prefetch)
                pv_A = pvps.tile([65, 512], f32, tag="pv_A")
                pv_B = pvps.tile([65, 512], f32, tag="pv_B")

                def scores(st):
                    ks = slice(b * S + st * 128, b * S + (st + 1) * 128)
                    ps = scps.tile([128, 1024], f32, tag="ps_sc")
                    nc.tensor.matmul(ps[:, 0:512], kt_sb[0:64, ks],
                                     qt_sb[0:64, tg:tg + 512],
                                     start=True, stop=True)
                    nc.tensor.matmul(ps[:, 512:1024], kt_sb[64:128, ks],
                                     qt_sb[64:128, tg:tg + 512],
                                     start=True, stop=True)
                    return ps

                ps_next = scores(0)
                for st in range(NST):
                    ps_cur = ps_next
                    if st < NST - 1:
                        if reqs and st in reqs:
                            feeder.require(*reqs[st])
                        ps_next = scores(st + 1)
                    pr = prsb.tile([128, 1024], bf16, tag="pr")
                    nc.scalar.activation(pr[:], ps_cur[:], AF.Exp)
                    pt = prsb.tile([128, 1024], bf16, tag="pt")
                    nc.vector.tensor_tensor(pt[:, 0:512], pr[:, 0:512],
                                            bias_t[:, st, :], MULT)
                    nc.vector.tensor_tensor(pt[:, 512:1024], pr[:, 512:1024],
                                            bias_t[:, st, :], MULT)
                    if reqs_pv and st in reqs_pv:
                        feeder.require(*reqs_pv[st])
                    stg = b * NST + st
                    nc.tensor.matmul(pv_A[:], v_sb[:, stg, 0:65],
                                     pt[:, 0:512],
                                     start=(st == 0), stop=(st == NST - 1))
                    nc.tensor.matmul(pv_B[:], v_sb[:, stg, 65:130],
                                     pt[:, 512:1024],
                                     start=(st == 0), stop=(st == NST - 1))
                    feeder.feed(budget)
                at_t = evsb.tile([64, 2, 512], bf16, tag="atf")
                at_q[(b, tcc)] = at_t
                nc.vector.tensor_copy(at_t[:, 0, :], pv_A[0:64, :])
                nc.vector.tensor_copy(at_t[:, 1, :], pv_B[0:64, :])
                dn = evsb.tile([65, 2, 512], f32, tag="dn")
                nc.vector.tensor_copy(dn[64:65, 0, :], pv_A[64:65, :])
                nc.vector.tensor_copy(dn[64:65, 1, :], pv_B[64:65, :])
                bi = 0 if b == 0 else (1 if tcc < 2 else 2)
                nc.sync.dma_start(
                    a2a_in[bi][j, 0:128, :].rearrange("(t p) f -> p t f", t=2),
                    at_t[:])
                nc.sync.dma_start(a2a_in[bi][j, 128:132, :],
                                  dn[64:65, :, :].bitcast(bf16))

            # ---------------- output projection ----------------
            def op_fence(anchor_ap):
                # cap scheduler hoisting: den_rv's gather (and thus recip)
                # cannot be reordered above the anchor's producer, so the
                # DVE/gpsimd queues never head-of-line block on a collective
                nc.vector.tensor_copy(den_rv[0:1, 0:8], anchor_ap)

            def op_gather(bi):
                dsp = den_rv[:].rearrange("(c t) f -> c t f", t=2)
                for u in range(2):
                    nc.gpsimd.dma_start(
                        dsp[:, u, :],
                        a2a_out[bi][:, 128 + 2 * u:130 + 2 * u, :]
                        .bitcast(f32).rearrange("c t f -> c (t f)"))
                nc.gpsimd.dma_start(
                    attn_rv[0:64, :, :],
                    a2a_out[bi][:, 0:64, :].rearrange("c p f -> p c f"))
                nc.gpsimd.dma_start(
                    attn_rv[64:128, :, :],
                    a2a_out[bi][:, 64:128, :].rearrange("c p f -> p c f"))
                nc.vector.reciprocal_approx_fast(rcp_sb[:], den_rv[:])
                nc.vector.tensor_copy(rcp_bf[:], rcp_sb[:])

            def op_compute(out_t, acc):
                for oc in range(8):
                    rg_ps = prps.tile([128, 512], f32, tag="ps_pr")
                    nc.tensor.matmul(rg_ps[:],
                                     sel_sb[:, oc * 128:(oc + 1) * 128],
                                     rcp_bf[:], start=True, stop=True)
                    nc.vector.tensor_tensor(attn_n[:, oc, :],
                                            attn_rv[:, oc, :], rg_ps[:],
                                            MULT)
                for do in range(8):
                    ps_o = prps.tile([128, 512], f32, tag="ps_pr")
                    for oc in range(8):
                        nc.tensor.matmul(
                            ps_o[:], wo_sb[:, oc, do * 128:(do + 1) * 128],
                            attn_n[:, oc, :],
                            start=(oc == 0), stop=(oc == 7))
                    nc.vector.tensor_scalar_add(acc[:, do, :], ps_o[:],
                                                bo_sb[:, do:do + 1])
                    # flush this 128-row slab immediately (bf16, small DMA)
                    nc.sync.dma_start(
                        out_t.ap()[do * 128:(do + 1) * 128, :],
                        acc[:, do, :])

            # ================= emission =================
            # minimal head: project tc0 of K, Q, V for batch 0 inline.
            # sync-queue DMA order is strictly need-order: wk/bk (already
            # queued), xk00, wq/bq, xq00, then the first bias pieces.
            # V's weights + x ride the scalar queue in parallel.
            xk00 = x_dma(xk, 0, 0)
            nc.sync.dma_start(wq_sb[:], wq_t.ap())
            nc.sync.dma_start(bq_sb[:], bq_d.ap())
            xv00 = x_dma(xv, 0, 0, eng=nc.scalar)
            xq00 = x_dma(xq, 0, 0)
            for it in kq_proj_tc(xk00, wk_sb, bk_sb, kt_sb, 0, 0, "K00"):
                it[2]()
            for it in kq_proj_tc(xq00, wq_sb, bq_sb, qt_sb, 0, 0, "Q00"):
                it[2]()
            for it in v_proj_tc(xv00, 0, 0, "V00"):
                it[2]()

            # everything else goes through the feeder: x-load DMAs run two
            # groups ahead of their matmuls so the boundary never starves
            def load_wo():
                nc.gpsimd.dma_start(wo_sb[:], wo_t.ap())
                nc.gpsimd.dma_start(bo_sb[:], bo_d.ap())
                nc.gpsimd.dma_start(sel_sb[:], sel_d.ap())

            groups = []
            for tcg in (1, 2, 3):
                groups.append(tc_thunks("K", 0, tcg))
                groups.append(tc_thunks("V", 0, tcg))
            for tcg in (1, 2, 3):
                groups.append(tc_thunks("Q", 0, tcg))
            for tcg in range(4):
                groups.append(tc_thunks("K", 1, tcg))
                groups.append(tc_thunks("V", 1, tcg))
                groups.append(tc_thunks("Q", 1, tcg))
            def cache_flush(b):
                def f():
                    sl = slice(b * S, (b + 1) * S)
                    nc.gpsimd.dma_start(kt_out.ap()[:, sl], kt_sb[:, sl])
                    nc.gpsimd.dma_start(
                        v_out.ap()[:, b * NST:(b + 1) * NST, :],
                        v_sb[:, b * NST:(b + 1) * NST, :])
                return (0, None, f)

            AHEAD = 3
            for i in range(AHEAD):
                feeder.add(groups[i][0])
            for i, g in enumerate(groups):
                if i + AHEAD < len(groups):
                    feeder.add(groups[i + AHEAD][0])
                feeder.add(g[1])
                if i == 13:
                    feeder.add((0, None, load_wo))
            feeder.add(cache_flush(0))

            # attention batch 0; block 0 requires K/V chunks just in time.
            # xK01/xV01 ride the scalar queue; bias00 pieces go on sync right
            # after xq00 so st0's bias lands before the first exp needs it
            feeder._pop_one()   # xK01 dma (scalar)
            feeder._pop_one()   # xV01 dma (scalar)
            prefetch_bias(0, 0, pieces=2)
            feeder.feed(2500)
            prefetch_bias(0, 1)
            attn_block(0, 0, budget=275,
                       reqs={3: ("K01",), 7: ("K02",), 11: ("K03",)},
                       reqs_pv={4: ("V01",), 8: ("V02",), 12: ("V03",)},
                       prefetch=None)
            for tcc in range(1, 4):
                feeder.require(f"Q0{tcc}")
                nxt = (0, tcc + 1) if tcc < 3 else (1, 0)
                attn_block(0, tcc, budget=275, prefetch=nxt)

            # release only batch-1's first chunks before the collective; the
            # rest streams just-in-time into the batch-1 attention windows
            def a2a(bi):
                nc.gpsimd.collective_compute(
                    "AllToAll", mybir.AluOpType.bypass, replica_groups=rg,
                    ins=[a2a_in[bi][:].opt()], outs=[a2a_out[bi][:].opt()])

            feeder.require("K10", "V10", "Q10")
            a2a(0)

            attn_block(1, 0, budget=350,
                       reqs={3: ("K11",), 7: ("K12",), 11: ("K13",)},
                       reqs_pv={4: ("V11",), 8: ("V12",), 12: ("V13",)},
                       prefetch=(1, 1))
            # batch-0 gathers mid-stream, fenced on block (1,0)'s staging so
            # the scheduler cannot hoist them (or recip) above attention work
            op_fence(at_q[(1, 0)][0:1, 0, 0:8])
            op_gather(0)
            feeder.require("Q11")
            attn_block(1, 1, budget=350, prefetch=(1, 2))
            a2a(1)   # b1 blocks 0-1 -> cores 4,5; streams under blocks 2-3
            feeder.require("Q12")
            attn_block(1, 2, budget=350, prefetch=(1, 3))
            feeder.require("Q13")
            attn_block(1, 3, budget=350, prefetch=None)
            feeder.drain()

            op_compute(out_t0, out_accs[0])
            op_fence(at_q[(1, 2)][0:1, 0, 0:8])
            op_gather(1)
            op_compute(out_t1a, out_accs[1])
            a2a(2)   # b1 blocks 2-3 -> cores 6,7 (small tail exposure)
            op_fence(out_accs[1][0:1, 7, 0:8])
            op_gather(2)
            op_compute(out_t1b, out_accs[2])
            # batch-1 cache flush last on gpsimd: no consumer, just must
            # land before kernel end (overlaps the batch-1 outproj)
            cache_flush(1)[2]()

    return nc


def _get_nc():
    if "nc" not in _CACHE:
        nc = _build_nc()
        if not nc.is_finalized():
            nc.finalize()
        _CACHE["nc"] = nc
    return _CACHE["nc"]


def _chunkx(xt):
    # [D, T] -> [8 chunks, 128 p, 8 c, 512 t]; partition rows contiguous
    return np.ascontiguousarray(
        xt.reshape(8, 128, 8, 512).transpose(2, 1, 0, 3))


def _wfold(w):
    # [D, M] -> [128 p, 8 c, M]
    return np.ascontiguousarray(w.reshape(8, 128, -1).transpose(1, 0, 2))


def _prepare_in_maps(queries, keys, values, attn_bias, Wq, bq, Wk, bk, Wv, bv,
                     Wo, bo):
    f32 = np.float32
    xq_t = _chunkx(np.asarray(queries, f32).reshape(T, D).T.astype(BF))
    xk_t = _chunkx(np.asarray(keys, f32).reshape(T, D).T.astype(BF))
    xv_t = _chunkx(np.asarray(values, f32).reshape(T, D).T.astype(BF))
    eb = np.exp(np.transpose(np.asarray(attn_bias, f32)[:, 0],
                             (0, 2, 1))).astype(BF)
    ebias_t = np.ascontiguousarray(
        eb.reshape(B, NST, 128, 4, 512).transpose(0, 3, 2, 1, 4))

    Wq = np.asarray(Wq, f32); Wk = np.asarray(Wk, f32)
    Wv = np.asarray(Wv, f32); Wo = np.asarray(Wo, f32)
    bq = np.asarray(bq, f32); bk = np.asarray(bk, f32)
    bv = np.asarray(bv, f32); bo = np.asarray(bo, f32)

    scale = 1.0 / np.sqrt(np.float32(DK))
    wo_t = _wfold(Wo.T.astype(BF))
    bo_f = np.ascontiguousarray(bo.reshape(8, 128).T.astype(f32))
    sel = np.zeros((H, D), np.float32)
    for o in range(D):
        sel[o // DK, o] = 1.0
    in_maps = []
    for c in range(NCORES):
        sl = slice(c * OC, (c + 1) * OC)
        in_maps.append({
            "xq_t": xq_t, "xk_t": xk_t, "xv_t": xv_t, "ebias_t": ebias_t,
            "wq_t": _wfold((Wq[sl] * scale).T.astype(BF)),
            "wk_t": _wfold(Wk[sl].T.astype(BF)),
            "wv_t": _wfold(Wv[sl].T.astype(BF)),
            "wo_t": wo_t,
            "bq_r": np.ascontiguousarray(
                (bq[sl] * scale).reshape(1, OC)).astype(BF),
            "bk_r": np.ascontiguousarray(bk[sl].reshape(1, OC)).astype(BF),
            "bv_r": np.ascontiguousarray(bv[sl].reshape(1, OC)).astype(BF),
            "bo_f": bo_f,
            "sel": sel.astype(BF),
        })
    return in_maps


def _run(in_maps, trace=False):
    from concourse.bass_utils import run_bass_kernel_spmd

    nc = _get_nc()
    return run_bass_kernel_spmd(nc, in_maps, core_ids=list(range(NCORES)),
                                trace=trace)


def _assemble(results):
    out_full = np.empty((T, D), np.float32)
    k_full = np.empty((T, D), np.float32)
    v_full = np.empty((T, D), np.float32)
    for c in range(NCORES):
        r = results[c]
        k_full[:, c * OC:(c + 1) * OC] = np.asarray(r["kt_out"], np.float32).T
        vo = np.asarray(r["v_out"], np.float32).transpose(1, 0, 2).reshape(
            T, 130)
        v_full[:, c * OC:c * OC + 64] = vo[:, 0:64]
        v_full[:, c * OC + 64:(c + 1) * OC] = vo[:, 65:129]
        ot = r["out_t0"] if c < 4 else (
            r["out_t1a"] if c < 6 else r["out_t1b"])
        out_full[c * TSL:(c + 1) * TSL, :] = np.asarray(ot, np.float32).T
    return (out_full.reshape(B, S, D), k_full.reshape(B, S, D),
            v_full.reshape(B, S, D))


def kernel(**inputs):
    in_maps = _prepare_in_maps(**inputs)
    res = _run(in_maps, trace=False)
    return _assemble(res.results)

